# revision 14
# baseline (speedup 1.0000x reference)
"""Trainium2 Bass kernel for nn_LocalMixer: grouped 16x16 mixing conv.

out[b, h, t*16+go] = sum_gi W[h, go, gi] * x[b, h, t*16+gi]

Scheme: shard HIDDEN across the 8 cores (64 channels each, all 256 batches).
Per core, per batch-half of 128 b:
  1. Load natural tiles [(b2,h64), seq256] from HBM -- fully contiguous DMA.
  2. PE-transpose each 128x128 half into PSUM [(t,gi), (b2,h)], copy into a
     staging buffer XT[half] (copies alternate between DVE and ACT engines).
  3. x-stationary matmul per channel h: lhsT = 128 b-columns of XT for h,
     rhs = kron(I8, W[h].T) so PSUM result is [b, (t,go)] -- already the
     natural output layout; no second transpose.
  4. Copy PSUM -> SBUF out tile [b128, seq256], contiguous DMA store.

DMA is fully contiguous both directions (the roofline for this problem);
PE does 512 transposes + 256 matmuls; PSUM->SBUF copy traffic is split
between the vector and scalar engines.
"""

import numpy as np

B = 256
HIDDEN = 512
BLOCK = 16
GROUPS = 16
SEQ = BLOCK * GROUPS  # 256
N_CORES = 8
H_CORE = HIDDEN // N_CORES  # 64 hidden channels per core
NB = 2  # batch macro-tiles of 128
BP = 64  # b-pairs per macro-tile

_cached = None


def _build_bass():
    import concourse.mybir as mybir
    from concourse import bacc
    from concourse.tile import TileContext

    f32 = mybir.dt.float32
    nc = bacc.Bacc()
    x = nc.declare_dram_parameter("x", [B, H_CORE, SEQ], f32, isOutput=False)
    wk = nc.declare_dram_parameter(
        "wk", [128, H_CORE * 128 + 128], f32, isOutput=False
    )
    y = nc.declare_dram_parameter("y", [B, H_CORE, SEQ], f32, isOutput=True)

    with TileContext(nc) as tc:
        with (
            tc.tile_pool(name="wpool", bufs=1) as wpool,
            tc.tile_pool(name="natpool", bufs=6) as natpool,
            tc.tile_pool(name="xtpool", bufs=2) as xtpool,
            tc.tile_pool(name="obpool", bufs=6) as obpool,
            tc.tile_pool(name="pst", bufs=4, space="PSUM") as pst,
            tc.tile_pool(name="psm", bufs=4, space="PSUM") as psm,
        ):
            wk_all = wpool.tile([128, H_CORE * 128 + 128], f32)
            nc.sync.dma_start(out=wk_all, in_=wk[:, :])
            ident = wk_all[:, H_CORE * 128 : H_CORE * 128 + 128]

            for bb in range(NB):
                xt0 = xtpool.tile([128, BP * 128], f32, tag="xt0")
                xt1 = xtpool.tile([128, BP * 128], f32, tag="xt1")
                xts = [xt0, xt1]
                # ---- load + transpose phase ----
                for bp in range(BP):
                    nat = natpool.tile([128, SEQ], f32)
                    b0 = bb * 128 + bp * 2
                    nc.sync.dma_start(out=nat, in_=x[b0 : b0 + 2, :, :])
                    for half in range(2):
                        tp = pst.tile([128, 128], f32)
                        nc.tensor.transpose(
                            tp, nat[:, half * 128 : (half + 1) * 128], ident
                        )
                        dst = xts[half][:, bp * 128 : (bp + 1) * 128]
                        if (bp + half) % 2 == 0:
                            nc.vector.tensor_copy(out=dst, in_=tp)
                        else:
                            nc.scalar.copy(dst, tp)
                # ---- matmul + store phase ----
                for h in range(H_CORE):
                    ob = obpool.tile([128, SEQ], f32)
                    for half in range(2):
                        ps = psm.tile([128, 128], f32)
                        # columns {bp*128 + b2*64 + h} -> M order = b
                        lhsT = xts[half].rearrange(
                            "p (bp b2 h) -> p bp b2 h", b2=2, h=H_CORE
                        )[:, :, :, h]
                        nc.tensor.matmul(
                            ps,
                            lhsT,
                            wk_all[:, h * 128 : (h + 1) * 128],
                            start=True,
                            stop=True,
                        )
                        dst = ob[:, half * 128 : (half + 1) * 128]
                        if (h + half) % 2 == 0:
                            nc.vector.tensor_copy(out=dst, in_=ps)
                        else:
                            nc.scalar.copy(dst, ps)
                    nc.sync.dma_start(
                        out=y[bb * 128 : (bb + 1) * 128, h, :], in_=ob
                    )

    nc.finalize()
    return nc


def _pack_weights(W: np.ndarray) -> np.ndarray:
    """Per-core wk [128, H_CORE*128 + 128]: kron(I8, W[h].T) blocks + I128."""
    eye8 = np.eye(8, dtype=np.float32)
    wks = np.empty((N_CORES, 128, H_CORE * 128 + 128), dtype=np.float32)
    for c in range(N_CORES):
        for h in range(H_CORE):
            Wt = W[c * H_CORE + h].T.astype(np.float32)  # [gi, go]
            wks[c, :, h * 128 : (h + 1) * 128] = np.kron(eye8, Wt)
        wks[c, :, H_CORE * 128 :] = np.eye(128, dtype=np.float32)
    return wks


def _get_bass():
    global _cached
    if _cached is None:
        _cached = _build_bass()
    return _cached


def kernel(x: np.ndarray, W: np.ndarray, _trace: bool = False):
    from concourse.bass_utils import run_bass_kernel_spmd

    nc = _get_bass()
    x = np.asarray(x, dtype=np.float32).reshape(B, HIDDEN, SEQ)
    wks = _pack_weights(np.asarray(W, dtype=np.float32))

    in_maps = []
    for c in range(N_CORES):
        xc = np.ascontiguousarray(x[:, c * H_CORE : (c + 1) * H_CORE, :])
        in_maps.append({"x": xc, "wk": wks[c]})

    res = run_bass_kernel_spmd(
        nc, in_maps, core_ids=list(range(N_CORES)), trace=_trace
    )
    out = np.concatenate([r["y"] for r in res.results], axis=1)
    out = out.reshape(B, HIDDEN, 1, SEQ)
    if _trace:
        kernel._last_results = res
    return out


# revision 23
# speedup vs baseline: 1.5423x; 1.5423x over previous
"""Trainium2 Bass kernel for nn_LocalMixer: grouped 16x16 mixing conv.

out[b, h, t*16+go] = sum_gi W[h, go, gi] * x[b, h, t*16+gi]

Scheme: shard HIDDEN across the 8 cores (64 channels each, all 256 batches).
Per core, per batch-half of 128 b:
  1. Load natural chunks [(b2,h64), bp_chunk*seq256] from HBM -- contiguous
     ~1 MiB DMAs.
  2. PE-transpose each 128x128 block into PSUM [(t,gi), (b2,h)], copy into a
     staging buffer XT[half] (half0 copies on DVE, half1 on ACT).
  3. x-stationary matmul per channel h: lhsT = 128 b-columns of XT for h,
     rhs = kron(I8, W[h].T), so PSUM comes out as [b, (t,go)] -- already the
     natural output layout; no second transpose.
  4. Copy PSUM -> SBUF out chunk [b128, 8h*seq256], contiguous ~1 MiB stores.

Matmul/transpose operands are bitcast to float32r (FP22-truncated single-pass
PE mode, ~4x faster than true fp32; accumulate stays fp32).
"""

import numpy as np

B = 256
HIDDEN = 512
BLOCK = 16
GROUPS = 16
SEQ = BLOCK * GROUPS  # 256
N_CORES = 8
H_CORE = HIDDEN // N_CORES  # 64 hidden channels per core
NB = 2  # batch macro-tiles of 128
BP = 64  # b-pairs per macro-tile
BPC = 8  # b-pairs per input DMA chunk
HC = 8  # h channels per output DMA chunk

_cached = None


def _build_bass():
    import concourse.mybir as mybir
    from concourse import bacc
    from concourse.tile import TileContext

    f32 = mybir.dt.float32
    f32r = mybir.dt.float32r
    nc = bacc.Bacc()
    x = nc.declare_dram_parameter("x", [B, H_CORE, SEQ], f32r, isOutput=False)
    wk = nc.declare_dram_parameter(
        "wk", [128, H_CORE * 128 + 128], f32r, isOutput=False
    )
    y = nc.declare_dram_parameter("y", [B, H_CORE, SEQ], f32, isOutput=True)

    with TileContext(nc) as tc:
        with (
            tc.tile_pool(name="wpool", bufs=1) as wpool,
            tc.tile_pool(name="natpool", bufs=2) as natpool,
            tc.tile_pool(name="xtpool", bufs=2) as xtpool,
            tc.tile_pool(name="obpool", bufs=2) as obpool,
            tc.tile_pool(name="pst", bufs=4, space="PSUM") as pst,
            tc.tile_pool(name="psm", bufs=4, space="PSUM") as psm,
        ):
            wk_all = wpool.tile([128, H_CORE * 128 + 128], f32r)
            nc.sync.dma_start(out=wk_all, in_=wk[:, :])
            ident = wk_all[:, H_CORE * 128 : H_CORE * 128 + 128]

            for bb in range(NB):
                xt0 = xtpool.tile([128, BP * 128], f32r, tag="xt0")
                xt1 = xtpool.tile([128, BP * 128], f32r, tag="xt1")
                xts = [xt0, xt1]
                # ---- load + transpose phase ----
                for bc in range(BP // BPC):
                    nat = natpool.tile([128, BPC * SEQ], f32r)
                    b0 = bb * 128 + bc * BPC * 2
                    # src x[b0 : b0+2*BPC, :, :] as (b2, h, bp, s); one DMA
                    # per b2 so the AP stays 3-dim.
                    xs = x[b0 : b0 + 2 * BPC, :, :].rearrange(
                        "(bp b2) h s -> bp b2 h s", b2=2
                    )
                    for b2 in range(2):
                        # dest partition h, free (bp, s); src enumerated
                        # (h, bp, s) to match, 1 KiB contiguous runs.
                        nc.sync.dma_start(
                            out=nat[b2 * 64 : (b2 + 1) * 64, :],
                            in_=xs[:, b2, :, :].transpose([1, 0, 2]),
                        )
                    for bpl in range(BPC):
                        bp = bc * BPC + bpl
                        for half in range(2):
                            tp = pst.tile([128, 128], f32r)
                            nc.tensor.transpose(
                                tp,
                                nat[:, bpl * SEQ + half * 128 : bpl * SEQ + (half + 1) * 128],
                                ident,
                            )
                            dst = xts[half][:, bp * 128 : (bp + 1) * 128]
                            if half == 0:
                                nc.vector.tensor_copy(out=dst, in_=tp)
                            else:
                                nc.scalar.copy(dst, tp)
                # ---- matmul + store phase ----
                for hc in range(H_CORE // HC):
                    ob = obpool.tile([128, HC * SEQ], f32)
                    for hl in range(HC):
                        h = hc * HC + hl
                        for half in range(2):
                            ps = psm.tile([128, 128], f32)
                            # columns {bp*128 + b2*64 + h} -> M order = b
                            lhsT = xts[half].rearrange(
                                "p (bp b2 h) -> p bp b2 h", b2=2, h=H_CORE
                            )[:, :, :, h]
                            nc.tensor.matmul(
                                ps,
                                lhsT,
                                wk_all[:, h * 128 : (h + 1) * 128],
                                start=True,
                                stop=True,
                            )
                            dst = ob[:, hl * SEQ + half * 128 : hl * SEQ + (half + 1) * 128]
                            if h % 2 == 0:
                                nc.vector.tensor_copy(out=dst, in_=ps)
                            else:
                                nc.scalar.copy(dst, ps)
                    nc.sync.dma_start(
                        out=y[bb * 128 : (bb + 1) * 128, hc * HC : (hc + 1) * HC, :],
                        in_=ob.rearrange("b (h s) -> b h s", s=SEQ),
                    )

    nc.finalize()
    return nc


def _pack_weights(W: np.ndarray) -> np.ndarray:
    """Per-core wk [128, H_CORE*128 + 128]: kron(I8, W[h].T) blocks + I128."""
    eye8 = np.eye(8, dtype=np.float32)
    wks = np.empty((N_CORES, 128, H_CORE * 128 + 128), dtype=np.float32)
    for c in range(N_CORES):
        for h in range(H_CORE):
            Wt = W[c * H_CORE + h].T.astype(np.float32)  # [gi, go]
            wks[c, :, h * 128 : (h + 1) * 128] = np.kron(eye8, Wt)
        wks[c, :, H_CORE * 128 :] = np.eye(128, dtype=np.float32)
    return wks


def _get_bass():
    global _cached
    if _cached is None:
        _cached = _build_bass()
    return _cached


def kernel(x: np.ndarray, W: np.ndarray, _trace: bool = False):
    from concourse.bass_utils import run_bass_kernel_spmd

    nc = _get_bass()
    x = np.asarray(x, dtype=np.float32).reshape(B, HIDDEN, SEQ)
    wks = _pack_weights(np.asarray(W, dtype=np.float32))

    in_maps = []
    for c in range(N_CORES):
        xc = np.ascontiguousarray(x[:, c * H_CORE : (c + 1) * H_CORE, :])
        in_maps.append({"x": xc, "wk": wks[c]})

    res = run_bass_kernel_spmd(
        nc, in_maps, core_ids=list(range(N_CORES)), trace=_trace
    )
    out = np.concatenate([r["y"] for r in res.results], axis=1)
    out = out.reshape(B, HIDDEN, 1, SEQ)
    if _trace:
        kernel._last_results = res
    return out


# revision 24
# speedup vs baseline: 1.9501x; 1.2644x over previous
"""Trainium2 Bass kernel for nn_LocalMixer: grouped 16x16 mixing conv.

out[b, h, t*16+go] = sum_gi W[h, go, gi] * x[b, h, t*16+gi]

Scheme: shard HIDDEN across the 8 cores (64 channels each, all 256 batches).
Per core, per batch-half of 128 b (partition dim = batch index everywhere):
  1. Load nat tiles [b128, (h16,s256)] -- fully contiguous 2 MiB DMAs
     (64 KiB per-partition rows, line-rate descriptors).
  2. PE-transpose nat[:, (h, s-half)] 128x128 blocks -> PSUM [(t,gi), b],
     copy (DVE/ACT alternating by h) into XT[half][:, h*128:(h+1)*128].
  3. x-stationary matmul per h: lhsT = XT[half][:, h-slice] (contiguous),
     rhs = kron(I8, W[h].T) -> PSUM [b, (t,go)] = natural output layout.
  4. Copy PSUM -> ob [b128, (h16,s256)], contiguous 2 MiB stores.

PE operands are float32r (FP22-truncated single-pass mode; fp32 accumulate).
"""

import numpy as np

B = 256
HIDDEN = 512
BLOCK = 16
GROUPS = 16
SEQ = BLOCK * GROUPS  # 256
N_CORES = 8
H_CORE = HIDDEN // N_CORES  # 64 hidden channels per core
NB = 2  # batch macro-tiles of 128
HSL = 16  # h channels per input/output DMA slice

_cached = None


def _build_bass():
    import concourse.mybir as mybir
    from concourse import bacc
    from concourse.tile import TileContext

    f32 = mybir.dt.float32
    f32r = mybir.dt.float32r
    nc = bacc.Bacc()
    x = nc.declare_dram_parameter("x", [B, H_CORE, SEQ], f32r, isOutput=False)
    wk = nc.declare_dram_parameter(
        "wk", [128, H_CORE * 128 + 128], f32r, isOutput=False
    )
    y = nc.declare_dram_parameter("y", [B, H_CORE, SEQ], f32, isOutput=True)

    with TileContext(nc) as tc:
        with (
            tc.tile_pool(name="wpool", bufs=1) as wpool,
            tc.tile_pool(name="natpool", bufs=2) as natpool,
            tc.tile_pool(name="xtpool", bufs=1) as xtpool,
            tc.tile_pool(name="obpool", bufs=2) as obpool,
            tc.tile_pool(name="pst", bufs=4, space="PSUM") as pst,
            tc.tile_pool(name="psm", bufs=4, space="PSUM") as psm,
        ):
            wk_all = wpool.tile([128, H_CORE * 128 + 128], f32r)
            nc.sync.dma_start(out=wk_all, in_=wk[:, :])
            ident = wk_all[:, H_CORE * 128 : H_CORE * 128 + 128]

            for bb in range(NB):
                xt0 = xtpool.tile([128, H_CORE * 128], f32r, tag="xt0")
                xt1 = xtpool.tile([128, H_CORE * 128], f32r, tag="xt1")
                xts = [xt0, xt1]
                # ---- load + transpose phase ----
                for hs in range(H_CORE // HSL):
                    nat = natpool.tile([128, HSL * SEQ], f32r)
                    # [b128, (h16, s256)] <- contiguous rows of x
                    nc.sync.dma_start(
                        out=nat,
                        in_=x[bb * 128 : (bb + 1) * 128, hs * HSL : (hs + 1) * HSL, :],
                    )
                    for hl in range(HSL):
                        h = hs * HSL + hl
                        for half in range(2):
                            tp = pst.tile([128, 128], f32r)
                            nc.tensor.transpose(
                                tp,
                                nat[:, hl * SEQ + half * 128 : hl * SEQ + (half + 1) * 128],
                                ident,
                            )
                            dst = xts[half][:, h * 128 : (h + 1) * 128]
                            if h % 2 == 0:
                                nc.vector.tensor_copy(out=dst, in_=tp)
                            else:
                                nc.scalar.copy(dst, tp)
                # ---- matmul + store phase ----
                for hs in range(H_CORE // HSL):
                    ob = obpool.tile([128, HSL * SEQ], f32)
                    for hl in range(HSL):
                        h = hs * HSL + hl
                        for half in range(2):
                            ps = psm.tile([128, 128], f32)
                            nc.tensor.matmul(
                                ps,
                                xts[half][:, h * 128 : (h + 1) * 128],
                                wk_all[:, h * 128 : (h + 1) * 128],
                                start=True,
                                stop=True,
                            )
                            dst = ob[:, hl * SEQ + half * 128 : hl * SEQ + (half + 1) * 128]
                            if hs % 2 == 0:
                                nc.vector.tensor_copy(out=dst, in_=ps)
                            else:
                                nc.scalar.copy(dst, ps)
                    nc.sync.dma_start(
                        out=y[bb * 128 : (bb + 1) * 128, hs * HSL : (hs + 1) * HSL, :],
                        in_=ob.rearrange("b (h s) -> b h s", s=SEQ),
                    )

    nc.finalize()
    return nc


def _pack_weights(W: np.ndarray) -> np.ndarray:
    """Per-core wk [128, H_CORE*128 + 128]: kron(I8, W[h].T) blocks + I128."""
    eye8 = np.eye(8, dtype=np.float32)
    wks = np.empty((N_CORES, 128, H_CORE * 128 + 128), dtype=np.float32)
    for c in range(N_CORES):
        for h in range(H_CORE):
            Wt = W[c * H_CORE + h].T.astype(np.float32)  # [gi, go]
            wks[c, :, h * 128 : (h + 1) * 128] = np.kron(eye8, Wt)
        wks[c, :, H_CORE * 128 :] = np.eye(128, dtype=np.float32)
    return wks


def _get_bass():
    global _cached
    if _cached is None:
        _cached = _build_bass()
    return _cached


def kernel(x: np.ndarray, W: np.ndarray, _trace: bool = False):
    from concourse.bass_utils import run_bass_kernel_spmd

    nc = _get_bass()
    x = np.asarray(x, dtype=np.float32).reshape(B, HIDDEN, SEQ)
    wks = _pack_weights(np.asarray(W, dtype=np.float32))

    in_maps = []
    for c in range(N_CORES):
        xc = np.ascontiguousarray(x[:, c * H_CORE : (c + 1) * H_CORE, :])
        in_maps.append({"x": xc, "wk": wks[c]})

    res = run_bass_kernel_spmd(
        nc, in_maps, core_ids=list(range(N_CORES)), trace=_trace
    )
    out = np.concatenate([r["y"] for r in res.results], axis=1)
    out = out.reshape(B, HIDDEN, 1, SEQ)
    if _trace:
        kernel._last_results = res
    return out


# revision 25
# speedup vs baseline: 2.0839x; 1.0686x over previous
"""Trainium2 Bass kernel for nn_LocalMixer: grouped 16x16 mixing conv.

out[b, h, t*16+go] = sum_gi W[h, go, gi] * x[b, h, t*16+gi]

Scheme: shard HIDDEN across the 8 cores (64 channels each, all 256 batches).
Per core, per batch-half of 128 b (partition dim = batch index everywhere):
  1. Load nat tiles [b128, (h8,s256)] fp16 -- contiguous 1 MiB reads with
     f32->fp16 cast during the DMA (SWDGE).
  2. PE-transpose nat[:, (h, s-half)] 128x128 blocks -> PSUM [(t,gi), b],
     copy (DVE/ACT alternating by h) into XT[half][:, h*128:(h+1)*128].
  3. x-stationary matmul per h: lhsT = XT[half][:, h-slice] (contiguous),
     rhs = kron(I8, W[h].T) fp16 -> PSUM fp32 [b, (t,go)] = natural output
     layout (no second transpose).
  4. Copy PSUM -> ob [b128, (h8,s256)] f32, contiguous 1 MiB stores.

fp16 operands run the PE at 2-byte speed (FP22 multiply, fp32 accumulate);
values here are O(1) so fp16 range is ample and error ~5e-4.
"""

import numpy as np

B = 256
HIDDEN = 512
BLOCK = 16
GROUPS = 16
SEQ = BLOCK * GROUPS  # 256
N_CORES = 8
H_CORE = HIDDEN // N_CORES  # 64 hidden channels per core
NB = 2  # batch macro-tiles of 128
HSL = 8  # h channels per input/output DMA slice

_cached = None


def _build_bass():
    import concourse.mybir as mybir
    from concourse import bacc
    from concourse.tile import TileContext

    f32 = mybir.dt.float32
    f16 = mybir.dt.float16
    nc = bacc.Bacc()
    x = nc.declare_dram_parameter("x", [B, H_CORE, SEQ], f32, isOutput=False)
    wk = nc.declare_dram_parameter("wk", [128, H_CORE * 128], f16, isOutput=False)
    ident = nc.declare_dram_parameter("ident", [128, 128], f16, isOutput=False)
    y = nc.declare_dram_parameter("y", [B, H_CORE, SEQ], f32, isOutput=True)

    with TileContext(nc) as tc:
        with (
            tc.tile_pool(name="idpool", bufs=1) as idpool,
            tc.tile_pool(name="wpool", bufs=1) as wpool,
            tc.tile_pool(name="natpool", bufs=3) as natpool,
            tc.tile_pool(name="xtpool", bufs=2) as xtpool,
            tc.tile_pool(name="obpool", bufs=3) as obpool,
            tc.tile_pool(name="pst", bufs=4, space="PSUM") as pst,
            tc.tile_pool(name="psm", bufs=4, space="PSUM") as psm,
        ):
            id_t = idpool.tile([128, 128], f16)
            nc.sync.dma_start(out=id_t, in_=ident[:, :])
            wk_all = wpool.tile([128, H_CORE * 128], f16)
            nc.sync.dma_start(out=wk_all, in_=wk[:, :])

            for bb in range(NB):
                xt0 = xtpool.tile([128, H_CORE * 128], f16, tag="xt0")
                xt1 = xtpool.tile([128, H_CORE * 128], f16, tag="xt1")
                xts = [xt0, xt1]
                # ---- load + transpose phase ----
                for hs in range(H_CORE // HSL):
                    nat = natpool.tile([128, HSL * SEQ], f16)
                    # [b128, (h8, s256)] <- contiguous f32 rows, cast to fp16
                    nc.gpsimd.dma_start(
                        out=nat,
                        in_=x[bb * 128 : (bb + 1) * 128, hs * HSL : (hs + 1) * HSL, :],
                    )
                    for hl in range(HSL):
                        h = hs * HSL + hl
                        for half in range(2):
                            tp = pst.tile([128, 128], f16)
                            nc.tensor.transpose(
                                tp,
                                nat[:, hl * SEQ + half * 128 : hl * SEQ + (half + 1) * 128],
                                id_t,
                            )
                            dst = xts[half][:, h * 128 : (h + 1) * 128]
                            if h % 2 == 0:
                                nc.vector.tensor_copy(out=dst, in_=tp)
                            else:
                                nc.scalar.copy(dst, tp)
                # ---- matmul + store phase ----
                for hs in range(H_CORE // HSL):
                    ob = obpool.tile([128, HSL * SEQ], f32)
                    for hl in range(HSL):
                        h = hs * HSL + hl
                        for half in range(2):
                            ps = psm.tile([128, 128], f32)
                            nc.tensor.matmul(
                                ps,
                                xts[half][:, h * 128 : (h + 1) * 128],
                                wk_all[:, h * 128 : (h + 1) * 128],
                                start=True,
                                stop=True,
                            )
                            dst = ob[:, hl * SEQ + half * 128 : hl * SEQ + (half + 1) * 128]
                            if hs % 2 == 0:
                                nc.vector.tensor_copy(out=dst, in_=ps)
                            else:
                                nc.scalar.copy(dst, ps)
                    nc.sync.dma_start(
                        out=y[bb * 128 : (bb + 1) * 128, hs * HSL : (hs + 1) * HSL, :],
                        in_=ob.rearrange("b (h s) -> b h s", s=SEQ),
                    )

    nc.finalize()
    return nc


def _pack_weights(W: np.ndarray) -> np.ndarray:
    """Per-core wk [128, H_CORE*128] fp16: kron(I8, W[h].T) blocks."""
    eye8 = np.eye(8, dtype=np.float32)
    wks = np.empty((N_CORES, 128, H_CORE * 128), dtype=np.float16)
    for c in range(N_CORES):
        for h in range(H_CORE):
            Wt = W[c * H_CORE + h].T.astype(np.float32)  # [gi, go]
            wks[c, :, h * 128 : (h + 1) * 128] = np.kron(eye8, Wt).astype(
                np.float16
            )
    return wks


def _get_bass():
    global _cached
    if _cached is None:
        _cached = _build_bass()
    return _cached


def kernel(x: np.ndarray, W: np.ndarray, _trace: bool = False):
    from concourse.bass_utils import run_bass_kernel_spmd

    nc = _get_bass()
    x = np.asarray(x, dtype=np.float32).reshape(B, HIDDEN, SEQ)
    wks = _pack_weights(np.asarray(W, dtype=np.float32))
    ident = np.eye(128, dtype=np.float16)

    in_maps = []
    for c in range(N_CORES):
        xc = np.ascontiguousarray(x[:, c * H_CORE : (c + 1) * H_CORE, :])
        in_maps.append({"x": xc, "wk": wks[c], "ident": ident})

    res = run_bass_kernel_spmd(
        nc, in_maps, core_ids=list(range(N_CORES)), trace=_trace
    )
    out = np.concatenate([r["y"] for r in res.results], axis=1)
    out = out.reshape(B, HIDDEN, 1, SEQ)
    if _trace:
        kernel._last_results = res
    return out
